# revision 83
# baseline (speedup 1.0000x reference)
"""FRFN forward kernel for 8 Trainium2 NeuronCores.

Sharding: pure data parallel over batch B=64 -> 8 batches per core.
The TVConv generated weight (1, CH, 9, H, W) is batch-independent, so
the whole weight path (3-conv+LN head AND the big final conv) is folded
into host-side input marshalling: each core DMAs the ready wgt tensor
(~5 MB bf16) instead of spending ~40us of PE recomputing it.

Channel packing: CH=1360 -> 11 tiles of 128 (vs 12 naively padded).
x1 channels [0,640) -> tiles 0-4, x2 channels [680,1320) -> tiles 5-9,
tile 10 holds both 40-wide tails (x1 tail at partitions 0-39, x2 tail
at 40-79); an SBUF->SBUF DMA re-aligns the x2 tail for the gate.

Per-channel-tile pipeline (steady state, cost-model ns; all three
compute engines run ~100% busy through the middle of the kernel):
  PE   : proj_in 8 matmuls (1307) + 8-9 ident-accumulation streams
         (~4700) into two 2-bank PSUM tiles
  DVE  : 6.x tap products (border-trimmed, 763-876 each) + merge add
         (non-gate iterations) + gate multiply (x2 iterations)
  Pool : corner taps 0,2 + 3 rows of tap 6 via tensor_mul (0.42
         ucode efficiency; SBUF-only: GPSIMD cannot access PSUM)
  ACT  : proj_in drains + tvacc drains (784-wide merged) + gelu
  DMA  : wgt stream (0.44 MB/tile) double-buffered 2 tiles ahead;
         NB the DMA-engine resource serializes transfers, so startup
         issues winx before wgt in strict priority order

h is stored unpadded (14x14): every product window stays inside the
valid interior because the out-regions are border-trimmed and all merge
dsts contain their srcs, so no pad is ever read. Each PSUM accumulation
group opens with the one full-region stream (tap 4); ident streams run
half-a-tile at a time so drains start early. Software pipelining:
idents trail products by 1 tile, gates by 2; the tail tile runs first
so its extra gate work hides mid-stream. The last tile merges deeper on
DVE, its idents follow its products in the same iteration, and
proj_out runs in three passes (tv5-7+tail early on PE slack, tv8 after
its gate, tv9 stops last) with the m=1 groups on the freed ident PSUM
ring so only stop matmuls + drains + output DMAs remain at the end.
"""

import numpy as np
import ml_dtypes
from contextlib import ExitStack

import concourse.bacc as bacc
import concourse.bass as bass
import concourse.mybir as mybir
import concourse.tile as tile
from concourse.bass_utils import run_bass_kernel_spmd

F32 = mybir.dt.float32
BF16 = mybir.dt.bfloat16
AF = mybir.ActivationFunctionType
OP = mybir.AluOpType

NCORES = 8
B = 64
BPC = B // NCORES          # 8 batches per core
DIM = 256
HID = 680
CH = 2 * HID               # 1360
NCT = 11                   # channel tiles: 5 x1 + 5 x2 + 1 tail(40+40)
CHP = NCT * 128            # 1408
HP = 14
NIJ = HP * HP              # 196
INTER = 64
NKPL = 9                   # 3x3 taps
NB2 = 2 * NIJ              # 392
NB4 = 4 * NIJ              # 784
EPS = 1e-5
WGT_CT = NKPL * NIJ        # 1764 wgt cols per channel tile

# taps whose products run on Pool (gpsimd) tensor_mul: corner taps have
# the smallest trimmed regions, fitting Pool's 0.42-efficiency rate.
# Tap 6 is split row-wise between Pool and DVE to equalize the three
# engines' per-iteration averages. (TensorScalarPtr would be 1.4x
# faster on Pool but neuronxcc rejects it on this engine.)
POOL_TAPS = (0, 2)
SPLIT_TAP = 6
SPLIT_ROWS = 3          # rows of tap 6's region computed on Pool
# the full-region tap that opens every PSUM accumulation group
ROOT_TAP = 4

# tail tile first so its extra gate work hides mid-stream; x1/x2 pairs
# interleaved so gelu(x1_k) is ready when x2_k finishes
CT_ORDER = [10, 0, 5, 1, 6, 2, 7, 3, 8, 4, 9]

LAG = 1          # idents trail products by 1 channel tile
GLAG = LAG + 1   # gate ops trail one further

_CACHE = {}
DEBUG_DUMP = False


def _valid(tap):
    """output (i0,i1,j0,j1) where tap's product is nonzero (pad elsewhere)"""
    di, dj = tap // 3, tap % 3
    i0, i1 = max(0, 1 - di), min(HP, HP + 1 - di)
    j0, j1 = max(0, 1 - dj), min(HP, HP + 1 - dj)
    return (i0, i1, j0, j1)


def _dve_gate_iter(idx):
    """does iteration idx finalize with a DVE gate multiply? (x2 tiles
    and the tail finalize on DVE, x1 tiles on ACT)"""
    fin = CT_ORDER[idx - GLAG] if idx >= GLAG else None
    return fin is not None and fin >= 5


def _merge_plan(idx):
    """DVE in-place merge adds (dst, src): only on iterations without a
    DVE gate multiply — on gate iterations t5 rides the PE instead.
    The last tile merges deeper: DVE is past its steady-state load
    there while everything downstream hangs off the PE ident streams"""
    if idx >= NCT - 1:
        return ((4, 5), (4, 3))
    if _dve_gate_iter(idx):
        return ()
    return ((4, 5),)


def _pool_taps(idx):
    return POOL_TAPS


def _build_nc():
    nc = bacc.Bacc("TRN2", target_bir_lowering=False)

    winxT = nc.dram_tensor("winxT", [DIM, CHP + BPC * NIJ], BF16,
                           kind="ExternalInput")
    wgtD = nc.dram_tensor("wgtD", [128, NCT * WGT_CT], BF16,
                          kind="ExternalInput")
    woutD = nc.dram_tensor("woutD", [128, 6, DIM], BF16, kind="ExternalInput")
    identD = nc.dram_tensor("identD", [128, 128], BF16, kind="ExternalInput")
    out_f = nc.dram_tensor("out_f", [DIM, BPC * NIJ], BF16,
                           kind="ExternalOutput")
    if DEBUG_DUMP:
        dbg_h = nc.dram_tensor("dbg_h", [128, NCT * BPC * NIJ], BF16,
                               kind="ExternalOutput")
        dbg_tv = nc.dram_tensor("dbg_tv", [128, NCT * BPC * NIJ], BF16,
                                kind="ExternalOutput")

    with tile.TileContext(nc) as tc, ExitStack() as ctx:
        persist = ctx.enter_context(tc.tile_pool(name="persist", bufs=1))
        wgtpool = ctx.enter_context(tc.tile_pool(name="wgtpool", bufs=3))
        prodpool = ctx.enter_context(tc.tile_pool(name="prodpool", bufs=3))
        latepool = ctx.enter_context(tc.tile_pool(name="latepool", bufs=3))
        rootpool = ctx.enter_context(tc.tile_pool(name="rootpool", bufs=3))
        gapool = ctx.enter_context(tc.tile_pool(name="gapool", bufs=2))
        outpool = ctx.enter_context(tc.tile_pool(name="outpool", bufs=2))
        ps_proj = ctx.enter_context(
            tc.tile_pool(name="ps_proj", bufs=2, space="PSUM"))
        ps_tv = ctx.enter_context(
            tc.tile_pool(name="ps_tv", bufs=2, space="PSUM"))

        # ---------------- persistent SBUF tensors ----------------
        h_sb = [persist.tile([128, BPC, HP, HP], BF16, name="t", tag=f"h{i}")
                for i in range(NCT)]
        tvacc = [persist.tile([128, BPC * NIJ], BF16, name="t", tag=f"tv{i}")
                 for i in range(NCT)]
        winx_sb = [persist.tile([128, CHP + BPC * NIJ], BF16, name="t",
                                tag=f"wx{i}") for i in range(2)]
        win_sb = [t[:, 0:CHP] for t in winx_sb]
        x_sb = [t[:, CHP:CHP + BPC * NIJ] for t in winx_sb]
        wo_sb = persist.tile([128, 6, DIM], BF16, name="t", tag="wo")
        ident = persist.tile([128, 128], BF16, name="t", tag="ident")
        x2t_al = persist.tile([128, BPC * NIJ], BF16, name="t", tag="x2t")



        wgt_tiles = {}

        def wgt_dma(ct, qeng):
            w = wgtpool.tile([128, NKPL, NIJ], BF16, name="t", tag="wg")
            qeng.dma_start(w[:], wgtD[:, WGT_CT * ct:WGT_CT * (ct + 1)])
            wgt_tiles[ct] = w

        # PSUM tiles are [128, 2, 512]: two full 2KB banks, one 392-col
        # matmul group per bank (a group crossing a bank boundary breaks
        # accumulation), drained in one strided ACT copy.
        def proj_in(ct):
            for hf in range(2):
                ps = ps_proj.tile([128, 2, 512], F32, name="t", tag="pj")
                for g in range(2):
                    xsl = slice(NB2 * (2 * hf + g), NB2 * (2 * hf + g + 1))
                    for kt in range(2):
                        nc.tensor.matmul(
                            ps[:, g, 0:NB2],
                            win_sb[kt][:, 128 * ct:128 * (ct + 1)],
                            x_sb[kt][:, xsl],
                            start=(kt == 0), stop=(kt == 1))
                dst = h_sb[ct][:, 4 * hf:4 * hf + 4, :, :].rearrange(
                    "p (a c) i j -> p a (c i j)", a=2, c=2)
                nc.scalar.activation(dst, ps[:, :, 0:NB2], AF.Copy)

        def products(ct, idx):
            """9 tap products over trimmed regions; Pool corners via stt,
            the rest on DVE; DVE merge adds per _merge_plan. Returns the
            ident streams with the full-region root first."""
            plan = _merge_plan(idx)
            dead = {s for _, s in plan}
            pool_taps = _pool_taps(idx)
            wgt_sb = wgt_tiles.pop(ct)
            prods = [None] * NKPL

            def emit(kpl):
                di, dj = kpl // 3, kpl % 3
                i0, i1, j0, j1 = _valid(kpl)
                if kpl == ROOT_TAP:
                    pp = rootpool
                elif kpl in (0, 2, 6, 7, 8):   # consumed late in ident order
                    pp = latepool
                else:
                    pp = prodpool
                prod = pp.tile([128, BPC * NIJ], BF16,
                               name="t", tag=f"prod{kpl}")
                pr = prod[:].rearrange(
                    "p (b i j) -> p b i j", b=BPC, i=HP, j=HP)

                def op(eng, r0, r1):
                    def run(b0, b1):
                        nb = b1 - b0
                        wgb = (wgt_sb[:, kpl, :]
                               .rearrange("p (i j) -> p i j", i=HP, j=HP)
                               [:, r0:r1, j0:j1].unsqueeze(1)
                               .broadcast_to((128, nb, r1 - r0,
                                              j1 - j0)))
                        hwin = h_sb[ct][:, b0:b1, r0 + di - 1:r1 + di - 1,
                                        j0 + dj - 1:j1 + dj - 1]
                        eng.tensor_mul(pr[:, b0:b1, r0:r1, j0:j1],
                                       hwin, wgb)
                    if idx == 0:
                        # pipeline-fill: emit per batch-half so products
                        # start as soon as half-A of h drains
                        run(0, 4)
                        run(4, BPC)
                    else:
                        run(0, BPC)

                if kpl == SPLIT_TAP:
                    op(nc.gpsimd, i0, i0 + SPLIT_ROWS)
                    op(nc.vector, i0 + SPLIT_ROWS, i1)
                elif kpl in pool_taps:
                    op(nc.gpsimd, i0, i1)
                else:
                    op(nc.vector, i0, i1)
                prods[kpl] = (prod, (i0, i1, j0, j1))

            def merge(dst, src):
                dt_, dreg = prods[dst]
                st_, sreg = prods[src]
                assert (dreg[0] <= sreg[0] and dreg[1] >= sreg[1]
                        and dreg[2] <= sreg[2] and dreg[3] >= sreg[3]), \
                    (dst, src, dreg, sreg)
                i0, i1, j0, j1 = sreg
                dv = dt_[:].rearrange("p (b i j) -> p b i j",
                                      b=BPC, i=HP, j=HP)[:, :, i0:i1, j0:j1]
                sv = st_[:].rearrange("p (b i j) -> p b i j",
                                      b=BPC, i=HP, j=HP)[:, :, i0:i1, j0:j1]
                nc.vector.tensor_add(dv, dv, sv)

            # pool first (slowest); on DVE: root + srcs with each merge
            # issued immediately so the root (the first ident stream)
            # frees as early as possible; then the remaining plain taps
            for kpl in pool_taps:
                emit(kpl)
            emit(ROOT_TAP)
            for dst, src in plan:
                emit(src)
                merge(dst, src)
            for kpl in range(NKPL):
                if (kpl not in pool_taps and kpl != ROOT_TAP
                        and kpl not in dead):
                    emit(kpl)

            plain = [k for k in range(NKPL)
                     if k not in dead and k != ROOT_TAP
                     and k not in pool_taps]
            pool_plain = [k for k in pool_taps
                          if k != ROOT_TAP and k not in dead]
            order = [ROOT_TAP] + plain + pool_plain
            assert prods[ROOT_TAP][1] == (0, HP, 0, HP)
            return [prods[k] for k in order]

        def idents(ct, streams, drain=True):
            """accumulate the remaining streams in PSUM via PE identity
            matmuls: two 2-bank tiles, 2 groups each, merged ACT drains.
            With drain=False the PSUM tiles are returned undrained for a
            consumer that reads PSUM directly (the final gate)."""
            # half (batches 0-3) fully accumulated and drained before
            # half (batches 4-7) starts: halves the drain latency and
            # lets the tail's downstream chain begin on half 0 early
            ns = len(streams)
            psts = []
            for hf in range(2):
                pst = ps_tv.tile([128, 2, 512], F32, name="t", tag="tv")
                psts.append(pst)
                for si, (p, reg) in enumerate(streams):
                    i0, i1, j0, j1 = reg
                    for g2 in range(2):
                        g = 2 * hf + g2
                        ps = pst[:, g2, 0:NB2]
                        if reg == (0, HP, 0, HP):
                            mov = p[:, NB2 * g:NB2 * (g + 1)]
                            dst = ps
                        else:
                            pw = p[:].rearrange("p (b i j) -> p b i j",
                                                b=BPC, i=HP, j=HP)
                            mov = pw[:, 2 * g:2 * g + 2, i0:i1, j0:j1]
                            sw = ps.rearrange(
                                "p (b i j) -> p b i j", b=2, i=HP, j=HP)
                            dst = sw[:, :, i0:i1, j0:j1]
                        nc.tensor.matmul(dst, ident[:], mov,
                                         start=(si == 0),
                                         stop=(si == ns - 1))
                if drain:
                    dst = tvacc[ct][:, NB4 * hf:NB4 * (hf + 1)].rearrange(
                        "p (a n) -> p a n", a=2)
                    nc.scalar.activation(dst, pst[:, :, 0:NB2], AF.Copy)
            return psts

        ga_tiles = {}

        def gate_gelu(i):
            ga = gapool.tile([128, BPC * NIJ], BF16, name="t", tag="ga")
            nc.scalar.activation(ga[:], tvacc[i][:], AF.Gelu)
            ga_tiles[i] = ga

        def gate_mult(i):
            ga = ga_tiles.pop(i)
            nc.vector.tensor_mul(tvacc[5 + i][:], ga[:], tvacc[5 + i][:])

        def gate_tail():
            # shift x2 tail (partitions 40:80) down to 0:40 via SBUF DMA
            nc.scalar.dma_start(x2t_al[0:40, :], tvacc[10][40:80, :])
            ga = gapool.tile([128, BPC * NIJ], BF16, name="t", tag="ga")
            nc.scalar.activation(ga[0:40, :], tvacc[10][0:40, :], AF.Gelu)
            nc.vector.tensor_mul(x2t_al[0:40, :], ga[0:40, :],
                                 x2t_al[0:40, :])

        def finalize(pct):
            if pct < 5:
                gate_gelu(pct)
            elif pct < 10:
                gate_mult(pct - 5)
            else:
                gate_tail()

        # proj_out: W_out @ gated, two passes per m-half. pass1 opens the
        # PSUM groups with the early-gated contraction tiles (tv5-7, the
        # tail); pass2 adds the late tiles (tv8, tv9) and drains.
        po_tiles = {}

        def proj_out_pass1a(m, pool=None, tag="pj"):
            # contraction tiles gated well before the tail: tv5-7 + tail
            for hf in range(2):
                ps = (pool or ps_proj).tile([128, 2, 512], F32,
                                            name="t", tag=tag)
                po_tiles[(m, hf)] = ps
                for g in range(2):
                    sl = ps[:, g, 0:NB2]
                    xsl = slice(NB2 * (2 * hf + g), NB2 * (2 * hf + g + 1))
                    for ki, kt in enumerate((0, 1, 2)):
                        nc.tensor.matmul(
                            sl,
                            wo_sb[:, kt, 128 * m:128 * (m + 1)],
                            tvacc[5 + kt][:, xsl],
                            start=(ki == 0), stop=False)
                    nc.tensor.matmul(
                        sl,
                        wo_sb[0:40, 5, 128 * m:128 * (m + 1)],
                        x2t_al[0:40, xsl],
                        start=False, stop=False)

        def proj_out_pass1b(m):
            # tv8, gated one finalize slot before the end
            for hf in range(2):
                ps = po_tiles[(m, hf)]
                for g in range(2):
                    xsl = slice(NB2 * (2 * hf + g), NB2 * (2 * hf + g + 1))
                    nc.tensor.matmul(
                        ps[:, g, 0:NB2],
                        wo_sb[:, 3, 128 * m:128 * (m + 1)],
                        tvacc[8][:, xsl],
                        start=False, stop=False)

        def proj_out_pass2(m):
            # m=1 finishes last: its PSUM drains ride the otherwise-idle
            # DVE and its halves DMA out separately so the hf0 transfer
            # overlaps the hf1 drain
            for hf in range(2):
                ps = po_tiles.pop((m, hf))
                for g in range(2):
                    sl = ps[:, g, 0:NB2]
                    xsl = slice(NB2 * (2 * hf + g), NB2 * (2 * hf + g + 1))
                    nc.tensor.matmul(
                        sl,
                        wo_sb[:, 4, 128 * m:128 * (m + 1)],
                        tvacc[9][:, xsl],
                        start=False, stop=True)
                ot = outpool.tile([128, NB4], BF16, name="t",
                                  tag=f"ot{m}{hf}")
                dst = ot[:].rearrange("p (a n) -> p a n", a=2)
                if m == 1:
                    nc.vector.tensor_copy(dst, ps[:, :, 0:NB2])
                else:
                    nc.scalar.activation(dst, ps[:, :, 0:NB2], AF.Copy)
                qe = nc.sync if (m + hf) % 2 == 0 else nc.scalar
                qe.dma_start(
                    out_f[128 * m:128 * (m + 1),
                          NB4 * hf:NB4 * (hf + 1)], ot[:])

        # ---------------- software-pipelined main loop ----------------
        # startup DMAs spread over both HWDGE queues, issued before the
        # ACT table prewarm so transfers overlap the table loads and the
        # first proj_in/products start as early as possible
        # DMA transfers serialize on the DMA-engine resource with ~0.5us
        # of latency between transfers, so issue big transfers in strict
        # priority order. winx1 (the last proj_in dependency) is split
        # so the first tile's half-A groups and products start before
        # half-B's x columns land.
        HS = CHP + NB4
        nc.sync.dma_start(winx_sb[0][:, 0:HS], winxT[0:128, 0:HS])
        nc.scalar.dma_start(winx_sb[1][:, 0:HS], winxT[128:256, 0:HS])
        wgt_dma(CT_ORDER[0], nc.sync)
        nc.scalar.dma_start(winx_sb[1][:, HS:], winxT[128:256, HS:])
        nc.sync.dma_start(winx_sb[0][:, HS:], winxT[0:128, HS:])
        wgt_dma(CT_ORDER[1], nc.scalar)

        warm = persist.tile([1, 1], F32, name="t", tag="warm")
        nc.gpsimd.memset(warm[:], 1.0)
        wsink = persist.tile([1, 1], F32, name="t", tag="wsink")
        for fn in (AF.Gelu, AF.Copy):
            nc.scalar.activation(wsink[:], warm[:], fn)

        nc.scalar.dma_start(wo_sb[:], woutD[:])
        nc.scalar.dma_start(ident[:], identD[:])
        state = {}

        for idx, ct in enumerate(CT_ORDER):
            if idx + 2 < NCT:
                wgt_dma(CT_ORDER[idx + 2], nc.sync)
            if idx == 0:
                proj_in(CT_ORDER[0])
                proj_in(CT_ORDER[1])
            if idx + 2 < NCT:
                proj_in(CT_ORDER[idx + 2])
            if idx >= LAG:
                idents(CT_ORDER[idx - LAG], state.pop(CT_ORDER[idx - LAG]))
            if idx == NCT - 2:
                # tv5-7 and the tail are gated by now: open the m=0
                # proj_out groups here, in PE slack
                proj_out_pass1a(0)
            if idx == NCT - 1:
                # tail compaction: the final tile's products go ahead of
                # the mult(3) finalize (which waits on ACT drains and
                # would head-block the DVE FIFO); its idents follow the
                # products directly; tv8 joins the open proj_out groups
                # last on the PE queue (it waits on mult(3) anyway)
                state[ct] = products(ct, idx)
                finalize(CT_ORDER[idx - GLAG])       # gate_mult(3)
                finalize(CT_ORDER[idx - GLAG + 1])   # gelu(4)
                pst9 = idents(ct, state.pop(ct), drain=False)
                # final gate_mult(4) split per half, reading the
                # undrained ident PSUM directly on the otherwise-idle
                # DVE: the ACT drains leave the critical chain entirely
                ga4 = ga_tiles.pop(4)
                for hf in range(2):
                    gsl = slice(NB4 * hf, NB4 * (hf + 1))
                    nc.vector.tensor_mul(
                        tvacc[9][:, gsl].rearrange("p (a n) -> p a n",
                                                   a=2),
                        ga4[:, gsl].rearrange("p (a n) -> p a n", a=2),
                        pst9[hf][:, :, 0:NB2])
                proj_out_pass1b(0)
                # m=1 groups open on the ident PSUM tiles freed by the
                # gate reads, so only the tv9 stop matmuls remain at
                # the very end
                proj_out_pass1a(1, pool=ps_tv, tag="tv")
                proj_out_pass1b(1)
            else:
                if idx >= GLAG:
                    finalize(CT_ORDER[idx - GLAG])
                state[ct] = products(ct, idx)

        if DEBUG_DUMP:
            for i in range(NCT):
                sl = slice(BPC * NIJ * i, BPC * NIJ * (i + 1))
                nc.sync.dma_start(
                    dbg_h[:, sl],
                    h_sb[i][:].rearrange("p b i j -> p (b i j)"))
                nc.sync.dma_start(dbg_tv[:, sl], tvacc[i][:])

        # ---------------- proj_out epilogue ----------------
        proj_out_pass2(0)
        proj_out_pass2(1)

    nc.compile()
    return nc


# channel map: padded slot (ct, cc) -> raw channel or -1
def _chan_map():
    m = np.full(CHP, -1, np.int64)
    for ct in range(5):
        m[128 * ct:128 * (ct + 1)] = np.arange(128 * ct, 128 * (ct + 1))
    for ct in range(5, 10):
        m[128 * ct:128 * (ct + 1)] = np.arange(
            HID + 128 * (ct - 5), HID + 128 * (ct - 4))
    m[1280:1320] = np.arange(640, 680)          # x1 tail
    m[1320:1360] = np.arange(HID + 640, HID + 680)  # x2 tail
    return m


def _host_wgt(inputs):
    """fp32 numpy eval of the whole weight path (3-conv LN head + final
    conv); returns wgt packed (128, NCT*9*196) bf16 in the padded
    channel-tile layout."""
    posi = np.asarray(inputs["posi_map"], np.float32)[0]       # (4,14,14)
    x = posi
    for wk, gk, bk in (("w0", "g0", "b0"), ("w1", "g1", "b1"),
                       ("w2", "g2", "b2")):
        w = np.asarray(inputs[wk], np.float32)
        g = np.asarray(inputs[gk], np.float32)
        b = np.asarray(inputs[bk], np.float32)
        C = x.shape[0]
        xp = np.zeros((C, HP + 2, HP + 2), np.float32)
        xp[:, 1:15, 1:15] = x
        P = np.empty((C, 3, 3, NIJ), np.float32)
        for di in range(3):
            for dj in range(3):
                P[:, di, dj, :] = xp[:, di:di + HP, dj:dj + HP].reshape(C, NIJ)
        y = (w.reshape(INTER, C * 9) @ P.reshape(C * 9, NIJ))
        y = y.reshape(INTER, HP, HP)
        mu = y.mean()
        var = y.var()
        y = (y - mu) / np.sqrt(var + EPS) * g + b
        x = np.maximum(y, 0.0)
    h3p = np.zeros((INTER, HP + 2, HP + 2), np.float32)
    h3p[:, 1:15, 1:15] = x
    p3 = np.empty((576, NIJ), np.float32)
    for kap in range(NKPL):
        di, dj = kap // 3, kap % 3
        p3[kap * INTER:(kap + 1) * INTER] = \
            h3p[:, di:di + HP, dj:dj + HP].reshape(INTER, NIJ)

    # final conv as gemm: wgt[c, kpl, ij] = sum_r wfT[r, kpl, c] p3[r, ij]
    wf = np.asarray(inputs["wf"], np.float32)
    wf5 = wf.reshape(CH, NKPL, INTER, 3, 3)
    wfT = wf5.transpose(3, 4, 2, 1, 0).reshape(576, NKPL, CH)
    wgt = np.tensordot(wfT, p3, axes=(0, 0))    # (NKPL, CH, NIJ)
    wgt = wgt.transpose(1, 0, 2)                # (CH, NKPL, NIJ)

    cmap = _chan_map()
    valid = cmap >= 0
    wgtPad = np.zeros((CHP, NKPL, NIJ), np.float32)
    wgtPad[valid] = wgt[cmap[valid]]
    wgtPad = wgtPad.reshape(NCT, 128, WGT_CT).transpose(1, 0, 2)
    return np.ascontiguousarray(
        wgtPad.reshape(128, NCT * WGT_CT)).astype(ml_dtypes.bfloat16)


def _pack_shared(inputs):
    W_in = np.asarray(inputs["W_in"], np.float32)
    W_out = np.asarray(inputs["W_out"], np.float32)
    cmap = _chan_map()
    valid = cmap >= 0

    winP = np.zeros((CHP, DIM), np.float32)
    winP[valid] = W_in[cmap[valid]]
    winT = np.ascontiguousarray(winP.T).astype(ml_dtypes.bfloat16)
    # x is appended per core in kernel() to form winxT

    # W_out stationary tiles: (128, 6, 256); tile kt<5 partitions p = gated
    # channel 128*kt+p; tile 5 partitions 0:40 = channels 640:680
    woP = np.zeros((128, 6, DIM), np.float32)
    for kt in range(5):
        woP[:, kt, :] = W_out[:, 128 * kt:128 * (kt + 1)].T
    woP[0:40, 5, :] = W_out[:, 640:680].T
    woutD = woP.astype(ml_dtypes.bfloat16)

    identD = np.eye(128, dtype=np.float32).astype(ml_dtypes.bfloat16)

    return dict(winT=winT, wgtD=_host_wgt(inputs), woutD=woutD,
                identD=identD)


def kernel(**inputs) -> np.ndarray:
    if "nc" not in _CACHE:
        _CACHE["nc"] = _build_nc()
    nc = _CACHE["nc"]

    x = np.asarray(inputs["x"], np.float32)     # (64, 256, 14, 14)
    shared = _pack_shared(inputs)

    in_maps = []
    for c in range(NCORES):
        xc = x[BPC * c:BPC * (c + 1)]           # (8, 256, 14, 14)
        xT = np.ascontiguousarray(
            xc.transpose(1, 0, 2, 3).reshape(DIM, BPC * NIJ)
        ).astype(ml_dtypes.bfloat16)
        m = dict(shared)
        winT = m.pop("winT")
        m["winxT"] = np.ascontiguousarray(
            np.concatenate([winT, xT], axis=1))
        in_maps.append(m)

    res = run_bass_kernel_spmd(nc, in_maps, list(range(NCORES)))
    outs = []
    for c in range(NCORES):
        o = np.asarray(res.results[c]["out_f"], np.float32)
        o = o.reshape(DIM, BPC, HP, HP)
        outs.append(o.transpose(1, 0, 2, 3))
    return np.ascontiguousarray(np.concatenate(outs, axis=0), dtype=np.float32)


# revision 84
# speedup vs baseline: 1.0013x; 1.0013x over previous
"""FRFN forward kernel for 8 Trainium2 NeuronCores.

Sharding: pure data parallel over batch B=64 -> 8 batches per core.
The TVConv generated weight (1, CH, 9, H, W) is batch-independent, so
the whole weight path (3-conv+LN head AND the big final conv) is folded
into host-side input marshalling: each core DMAs the ready wgt tensor
(~5 MB bf16) instead of spending ~40us of PE recomputing it.

Channel packing: CH=1360 -> 11 tiles of 128 (vs 12 naively padded).
x1 channels [0,640) -> tiles 0-4, x2 channels [680,1320) -> tiles 5-9,
tile 10 holds both 40-wide tails (x1 tail at partitions 0-39, x2 tail
at 40-79); an SBUF->SBUF DMA re-aligns the x2 tail for the gate.

Per-channel-tile pipeline (steady state, cost-model ns; all three
compute engines run ~100% busy through the middle of the kernel):
  PE   : proj_in 8 matmuls (1307) + 8-9 ident-accumulation streams
         (~4700) into two 2-bank PSUM tiles
  DVE  : 6.x tap products (border-trimmed, 763-876 each) + merge add
         (non-gate iterations) + gate multiply (x2 iterations)
  Pool : corner taps 0,2 + 3 rows of tap 6 via tensor_mul (0.42
         ucode efficiency; SBUF-only: GPSIMD cannot access PSUM)
  ACT  : proj_in drains + tvacc drains (784-wide merged) + gelu
  DMA  : wgt stream (0.44 MB/tile) double-buffered 2 tiles ahead;
         NB the DMA-engine resource serializes transfers, so startup
         issues winx before wgt in strict priority order

h is stored unpadded (14x14): every product window stays inside the
valid interior because the out-regions are border-trimmed and all merge
dsts contain their srcs, so no pad is ever read. Each PSUM accumulation
group opens with the one full-region stream (tap 4); ident streams run
half-a-tile at a time so drains start early. Software pipelining:
idents trail products by 1 tile, gates by 2; the tail tile runs first
so its extra gate work hides mid-stream. The last tile merges deeper on
DVE, its idents follow its products in the same iteration, and
proj_out runs in three passes (tv5-7+tail early on PE slack, tv8 after
its gate, tv9 stops last) with the m=1 groups on the freed ident PSUM
ring so only stop matmuls + drains + output DMAs remain at the end.
"""

import numpy as np
import ml_dtypes
from contextlib import ExitStack

import concourse.bacc as bacc
import concourse.bass as bass
import concourse.mybir as mybir
import concourse.tile as tile
from concourse.bass_utils import run_bass_kernel_spmd

F32 = mybir.dt.float32
BF16 = mybir.dt.bfloat16
AF = mybir.ActivationFunctionType
OP = mybir.AluOpType

NCORES = 8
B = 64
BPC = B // NCORES          # 8 batches per core
DIM = 256
HID = 680
CH = 2 * HID               # 1360
NCT = 11                   # channel tiles: 5 x1 + 5 x2 + 1 tail(40+40)
CHP = NCT * 128            # 1408
HP = 14
NIJ = HP * HP              # 196
INTER = 64
NKPL = 9                   # 3x3 taps
NB2 = 2 * NIJ              # 392
NB4 = 4 * NIJ              # 784
EPS = 1e-5
WGT_CT = NKPL * NIJ        # 1764 wgt cols per channel tile

# taps whose products run on Pool (gpsimd) tensor_mul: corner taps have
# the smallest trimmed regions, fitting Pool's 0.42-efficiency rate.
# Tap 6 is split row-wise between Pool and DVE to equalize the three
# engines' per-iteration averages. (TensorScalarPtr would be 1.4x
# faster on Pool but neuronxcc rejects it on this engine.)
POOL_TAPS = (0, 2)
SPLIT_TAP = 6
SPLIT_ROWS = 3          # rows of tap 6's region computed on Pool
# the full-region tap that opens every PSUM accumulation group
ROOT_TAP = 4

# tail tile first so its extra gate work hides mid-stream; x1/x2 pairs
# interleaved so gelu(x1_k) is ready when x2_k finishes
CT_ORDER = [10, 0, 5, 1, 6, 2, 7, 3, 8, 4, 9]

LAG = 1          # idents trail products by 1 channel tile
GLAG = LAG + 1   # gate ops trail one further

_CACHE = {}
DEBUG_DUMP = False


def _valid(tap):
    """output (i0,i1,j0,j1) where tap's product is nonzero (pad elsewhere)"""
    di, dj = tap // 3, tap % 3
    i0, i1 = max(0, 1 - di), min(HP, HP + 1 - di)
    j0, j1 = max(0, 1 - dj), min(HP, HP + 1 - dj)
    return (i0, i1, j0, j1)


def _dve_gate_iter(idx):
    """does iteration idx finalize with a DVE gate multiply? (x2 tiles
    and the tail finalize on DVE, x1 tiles on ACT)"""
    fin = CT_ORDER[idx - GLAG] if idx >= GLAG else None
    return fin is not None and fin >= 5


def _merge_plan(idx):
    """DVE in-place merge adds (dst, src): only on iterations without a
    DVE gate multiply — on gate iterations t5 rides the PE instead.
    The last tile merges deeper: DVE is past its steady-state load
    there while everything downstream hangs off the PE ident streams"""
    if idx >= NCT - 1:
        return ((4, 5), (4, 3))
    if _dve_gate_iter(idx):
        return ()
    return ((4, 5),)


def _pool_taps(idx):
    return POOL_TAPS


def _build_nc():
    nc = bacc.Bacc("TRN2", target_bir_lowering=False)

    winxT = nc.dram_tensor("winxT", [DIM, CHP + BPC * NIJ], BF16,
                           kind="ExternalInput")
    wgtD = nc.dram_tensor("wgtD", [128, NCT * WGT_CT], BF16,
                          kind="ExternalInput")
    woutD = nc.dram_tensor("woutD", [128, 6, DIM], BF16, kind="ExternalInput")
    identD = nc.dram_tensor("identD", [128, 128], BF16, kind="ExternalInput")
    out_f = nc.dram_tensor("out_f", [DIM, BPC * NIJ], BF16,
                           kind="ExternalOutput")
    if DEBUG_DUMP:
        dbg_h = nc.dram_tensor("dbg_h", [128, NCT * BPC * NIJ], BF16,
                               kind="ExternalOutput")
        dbg_tv = nc.dram_tensor("dbg_tv", [128, NCT * BPC * NIJ], BF16,
                                kind="ExternalOutput")

    with tile.TileContext(nc) as tc, ExitStack() as ctx:
        persist = ctx.enter_context(tc.tile_pool(name="persist", bufs=1))
        wgtpool = ctx.enter_context(tc.tile_pool(name="wgtpool", bufs=3))
        prodpool = ctx.enter_context(tc.tile_pool(name="prodpool", bufs=3))
        latepool = ctx.enter_context(tc.tile_pool(name="latepool", bufs=3))
        rootpool = ctx.enter_context(tc.tile_pool(name="rootpool", bufs=3))
        gapool = ctx.enter_context(tc.tile_pool(name="gapool", bufs=2))
        outpool = ctx.enter_context(tc.tile_pool(name="outpool", bufs=2))
        ps_proj = ctx.enter_context(
            tc.tile_pool(name="ps_proj", bufs=2, space="PSUM"))
        ps_tv = ctx.enter_context(
            tc.tile_pool(name="ps_tv", bufs=2, space="PSUM"))

        # ---------------- persistent SBUF tensors ----------------
        h_sb = [persist.tile([128, BPC, HP, HP], BF16, name="t", tag=f"h{i}")
                for i in range(NCT)]
        tvacc = [persist.tile([128, BPC * NIJ], BF16, name="t", tag=f"tv{i}")
                 for i in range(NCT)]
        winx_sb = [persist.tile([128, CHP + BPC * NIJ], BF16, name="t",
                                tag=f"wx{i}") for i in range(2)]
        win_sb = [t[:, 0:CHP] for t in winx_sb]
        x_sb = [t[:, CHP:CHP + BPC * NIJ] for t in winx_sb]
        wo_sb = persist.tile([128, 6, DIM], BF16, name="t", tag="wo")
        ident = persist.tile([128, 128], BF16, name="t", tag="ident")
        x2t_al = persist.tile([128, BPC * NIJ], BF16, name="t", tag="x2t")



        wgt_tiles = {}

        def wgt_dma(ct, qeng):
            w = wgtpool.tile([128, NKPL, NIJ], BF16, name="t", tag="wg")
            qeng.dma_start(w[:], wgtD[:, WGT_CT * ct:WGT_CT * (ct + 1)])
            wgt_tiles[ct] = w

        # PSUM tiles are [128, 2, 512]: two full 2KB banks, one 392-col
        # matmul group per bank (a group crossing a bank boundary breaks
        # accumulation), drained in one strided ACT copy.
        def proj_in(ct):
            for hf in range(2):
                ps = ps_proj.tile([128, 2, 512], F32, name="t", tag="pj")
                for g in range(2):
                    xsl = slice(NB2 * (2 * hf + g), NB2 * (2 * hf + g + 1))
                    for kt in range(2):
                        nc.tensor.matmul(
                            ps[:, g, 0:NB2],
                            win_sb[kt][:, 128 * ct:128 * (ct + 1)],
                            x_sb[kt][:, xsl],
                            start=(kt == 0), stop=(kt == 1))
                dst = h_sb[ct][:, 4 * hf:4 * hf + 4, :, :].rearrange(
                    "p (a c) i j -> p a (c i j)", a=2, c=2)
                nc.scalar.activation(dst, ps[:, :, 0:NB2], AF.Copy)

        def products(ct, idx):
            """9 tap products over trimmed regions; Pool corners via stt,
            the rest on DVE; DVE merge adds per _merge_plan. Returns the
            ident streams with the full-region root first."""
            plan = _merge_plan(idx)
            dead = {s for _, s in plan}
            pool_taps = _pool_taps(idx)
            wgt_sb = wgt_tiles.pop(ct)
            prods = [None] * NKPL

            def emit(kpl):
                di, dj = kpl // 3, kpl % 3
                i0, i1, j0, j1 = _valid(kpl)
                if kpl == ROOT_TAP:
                    pp = rootpool
                elif kpl in (0, 2, 6, 7, 8):   # consumed late in ident order
                    pp = latepool
                else:
                    pp = prodpool
                prod = pp.tile([128, BPC * NIJ], BF16,
                               name="t", tag=f"prod{kpl}")
                pr = prod[:].rearrange(
                    "p (b i j) -> p b i j", b=BPC, i=HP, j=HP)

                def op(eng, r0, r1):
                    def run(b0, b1):
                        nb = b1 - b0
                        wgb = (wgt_sb[:, kpl, :]
                               .rearrange("p (i j) -> p i j", i=HP, j=HP)
                               [:, r0:r1, j0:j1].unsqueeze(1)
                               .broadcast_to((128, nb, r1 - r0,
                                              j1 - j0)))
                        hwin = h_sb[ct][:, b0:b1, r0 + di - 1:r1 + di - 1,
                                        j0 + dj - 1:j1 + dj - 1]
                        eng.tensor_mul(pr[:, b0:b1, r0:r1, j0:j1],
                                       hwin, wgb)
                    if idx == 0:
                        # pipeline-fill: emit per batch-half so products
                        # start as soon as half-A of h drains
                        run(0, 4)
                        run(4, BPC)
                    else:
                        run(0, BPC)

                if kpl == SPLIT_TAP:
                    op(nc.gpsimd, i0, i0 + SPLIT_ROWS)
                    op(nc.vector, i0 + SPLIT_ROWS, i1)
                elif kpl in pool_taps:
                    op(nc.gpsimd, i0, i1)
                else:
                    op(nc.vector, i0, i1)
                prods[kpl] = (prod, (i0, i1, j0, j1))

            def merge(dst, src):
                dt_, dreg = prods[dst]
                st_, sreg = prods[src]
                assert (dreg[0] <= sreg[0] and dreg[1] >= sreg[1]
                        and dreg[2] <= sreg[2] and dreg[3] >= sreg[3]), \
                    (dst, src, dreg, sreg)
                i0, i1, j0, j1 = sreg
                dv = dt_[:].rearrange("p (b i j) -> p b i j",
                                      b=BPC, i=HP, j=HP)[:, :, i0:i1, j0:j1]
                sv = st_[:].rearrange("p (b i j) -> p b i j",
                                      b=BPC, i=HP, j=HP)[:, :, i0:i1, j0:j1]
                nc.vector.tensor_add(dv, dv, sv)

            # pool first (slowest); on DVE: root + srcs with each merge
            # issued immediately so the root (the first ident stream)
            # frees as early as possible; then the remaining plain taps
            for kpl in pool_taps:
                emit(kpl)
            emit(ROOT_TAP)
            for dst, src in plan:
                emit(src)
                merge(dst, src)
            for kpl in range(NKPL):
                if (kpl not in pool_taps and kpl != ROOT_TAP
                        and kpl not in dead):
                    emit(kpl)

            plain = [k for k in range(NKPL)
                     if k not in dead and k != ROOT_TAP
                     and k not in pool_taps]
            pool_plain = [k for k in pool_taps
                          if k != ROOT_TAP and k not in dead]
            order = [ROOT_TAP] + plain + pool_plain
            assert prods[ROOT_TAP][1] == (0, HP, 0, HP)
            return [prods[k] for k in order]

        def idents(ct, streams, drain=True):
            """accumulate the remaining streams in PSUM via PE identity
            matmuls: two 2-bank tiles, 2 groups each, merged ACT drains.
            With drain=False the PSUM tiles are returned undrained for a
            consumer that reads PSUM directly (the final gate)."""
            # half (batches 0-3) fully accumulated and drained before
            # half (batches 4-7) starts: halves the drain latency and
            # lets the tail's downstream chain begin on half 0 early
            ns = len(streams)
            psts = []
            for hf in range(2):
                pst = ps_tv.tile([128, 2, 512], F32, name="t", tag="tv")
                psts.append(pst)
                for si, (p, reg) in enumerate(streams):
                    i0, i1, j0, j1 = reg
                    for g2 in range(2):
                        g = 2 * hf + g2
                        ps = pst[:, g2, 0:NB2]
                        if reg == (0, HP, 0, HP):
                            mov = p[:, NB2 * g:NB2 * (g + 1)]
                            dst = ps
                        else:
                            pw = p[:].rearrange("p (b i j) -> p b i j",
                                                b=BPC, i=HP, j=HP)
                            mov = pw[:, 2 * g:2 * g + 2, i0:i1, j0:j1]
                            sw = ps.rearrange(
                                "p (b i j) -> p b i j", b=2, i=HP, j=HP)
                            dst = sw[:, :, i0:i1, j0:j1]
                        nc.tensor.matmul(dst, ident[:], mov,
                                         start=(si == 0),
                                         stop=(si == ns - 1))
                if drain:
                    dst = tvacc[ct][:, NB4 * hf:NB4 * (hf + 1)].rearrange(
                        "p (a n) -> p a n", a=2)
                    nc.scalar.activation(dst, pst[:, :, 0:NB2], AF.Copy)
            return psts

        ga_tiles = {}

        def gate_gelu(i):
            ga = gapool.tile([128, BPC * NIJ], BF16, name="t", tag="ga")
            nc.scalar.activation(ga[:], tvacc[i][:], AF.Gelu)
            ga_tiles[i] = ga

        def gate_mult(i):
            ga = ga_tiles.pop(i)
            nc.vector.tensor_mul(tvacc[5 + i][:], ga[:], tvacc[5 + i][:])

        def gate_tail():
            # shift x2 tail (partitions 40:80) down to 0:40 via SBUF DMA
            nc.scalar.dma_start(x2t_al[0:40, :], tvacc[10][40:80, :])
            ga = gapool.tile([128, BPC * NIJ], BF16, name="t", tag="ga")
            nc.scalar.activation(ga[0:40, :], tvacc[10][0:40, :], AF.Gelu)
            nc.vector.tensor_mul(x2t_al[0:40, :], ga[0:40, :],
                                 x2t_al[0:40, :])

        def finalize(pct):
            if pct < 5:
                gate_gelu(pct)
            elif pct < 10:
                gate_mult(pct - 5)
            else:
                gate_tail()

        # proj_out: W_out @ gated, two passes per m-half. pass1 opens the
        # PSUM groups with the early-gated contraction tiles (tv5-7, the
        # tail); pass2 adds the late tiles (tv8, tv9) and drains.
        po_tiles = {}

        def proj_out_pass1a(m, pool=None, tag="pj"):
            # contraction tiles gated well before the tail: tv5-7 + tail
            for hf in range(2):
                ps = (pool or ps_proj).tile([128, 2, 512], F32,
                                            name="t", tag=tag)
                po_tiles[(m, hf)] = ps
                for g in range(2):
                    sl = ps[:, g, 0:NB2]
                    xsl = slice(NB2 * (2 * hf + g), NB2 * (2 * hf + g + 1))
                    for ki, kt in enumerate((0, 1, 2)):
                        nc.tensor.matmul(
                            sl,
                            wo_sb[:, kt, 128 * m:128 * (m + 1)],
                            tvacc[5 + kt][:, xsl],
                            start=(ki == 0), stop=False)
                    nc.tensor.matmul(
                        sl,
                        wo_sb[0:40, 5, 128 * m:128 * (m + 1)],
                        x2t_al[0:40, xsl],
                        start=False, stop=False)

        def proj_out_pass1b(m):
            # tv8, gated one finalize slot before the end
            for hf in range(2):
                ps = po_tiles[(m, hf)]
                for g in range(2):
                    xsl = slice(NB2 * (2 * hf + g), NB2 * (2 * hf + g + 1))
                    nc.tensor.matmul(
                        ps[:, g, 0:NB2],
                        wo_sb[:, 3, 128 * m:128 * (m + 1)],
                        tvacc[8][:, xsl],
                        start=False, stop=False)

        def proj_out_pass2(m):
            # m=1 finishes last: its PSUM drains ride the otherwise-idle
            # DVE and its halves DMA out separately so the hf0 transfer
            # overlaps the hf1 drain
            for hf in range(2):
                ps = po_tiles.pop((m, hf))
                for g in range(2):
                    sl = ps[:, g, 0:NB2]
                    xsl = slice(NB2 * (2 * hf + g), NB2 * (2 * hf + g + 1))
                    nc.tensor.matmul(
                        sl,
                        wo_sb[:, 4, 128 * m:128 * (m + 1)],
                        tvacc[9][:, xsl],
                        start=False, stop=True)
                ot = outpool.tile([128, NB4], BF16, name="t",
                                  tag=f"ot{m}{hf}")
                dst = ot[:].rearrange("p (a n) -> p a n", a=2)
                if m == 1:
                    nc.vector.tensor_copy(dst, ps[:, :, 0:NB2])
                else:
                    nc.scalar.activation(dst, ps[:, :, 0:NB2], AF.Copy)
                qe = nc.sync if (m + hf) % 2 == 0 else nc.scalar
                qe.dma_start(
                    out_f[128 * m:128 * (m + 1),
                          NB4 * hf:NB4 * (hf + 1)], ot[:])

        # ---------------- software-pipelined main loop ----------------
        # startup DMAs spread over both HWDGE queues, issued before the
        # ACT table prewarm so transfers overlap the table loads and the
        # first proj_in/products start as early as possible
        # DMA transfers serialize on the DMA-engine resource with ~0.5us
        # of latency between transfers, so issue big transfers in strict
        # priority order. winx1 (the last proj_in dependency) is split
        # so the first tile's half-A groups and products start before
        # half-B's x columns land.
        HS = CHP + NB4
        nc.sync.dma_start(winx_sb[0][:], winxT[0:128, :])
        nc.scalar.dma_start(winx_sb[1][:, 0:HS], winxT[128:256, 0:HS])
        nc.scalar.dma_start(winx_sb[1][:, HS:], winxT[128:256, HS:])
        wgt_dma(CT_ORDER[0], nc.sync)
        wgt_dma(CT_ORDER[1], nc.scalar)

        warm = persist.tile([1, 1], F32, name="t", tag="warm")
        nc.gpsimd.memset(warm[:], 1.0)
        wsink = persist.tile([1, 1], F32, name="t", tag="wsink")
        for fn in (AF.Gelu, AF.Copy):
            nc.scalar.activation(wsink[:], warm[:], fn)

        nc.scalar.dma_start(wo_sb[:], woutD[:])
        nc.scalar.dma_start(ident[:], identD[:])
        state = {}

        for idx, ct in enumerate(CT_ORDER):
            if idx + 2 < NCT:
                wgt_dma(CT_ORDER[idx + 2], nc.sync)
            if idx == 0:
                proj_in(CT_ORDER[0])
                proj_in(CT_ORDER[1])
            if idx + 2 < NCT:
                proj_in(CT_ORDER[idx + 2])
            if idx >= LAG:
                idents(CT_ORDER[idx - LAG], state.pop(CT_ORDER[idx - LAG]))
            if idx == NCT - 2:
                # tv5-7 and the tail are gated by now: open the m=0
                # proj_out groups here, in PE slack
                proj_out_pass1a(0)
            if idx == NCT - 1:
                # tail compaction: the final tile's products go ahead of
                # the mult(3) finalize (which waits on ACT drains and
                # would head-block the DVE FIFO); its idents follow the
                # products directly; tv8 joins the open proj_out groups
                # last on the PE queue (it waits on mult(3) anyway)
                state[ct] = products(ct, idx)
                finalize(CT_ORDER[idx - GLAG])       # gate_mult(3)
                finalize(CT_ORDER[idx - GLAG + 1])   # gelu(4)
                pst9 = idents(ct, state.pop(ct), drain=False)
                # final gate_mult(4) split per half, reading the
                # undrained ident PSUM directly on the otherwise-idle
                # DVE: the ACT drains leave the critical chain entirely
                ga4 = ga_tiles.pop(4)
                for hf in range(2):
                    gsl = slice(NB4 * hf, NB4 * (hf + 1))
                    nc.vector.tensor_mul(
                        tvacc[9][:, gsl].rearrange("p (a n) -> p a n",
                                                   a=2),
                        ga4[:, gsl].rearrange("p (a n) -> p a n", a=2),
                        pst9[hf][:, :, 0:NB2])
                proj_out_pass1b(0)
                # m=1 groups open on the ident PSUM tiles freed by the
                # gate reads, so only the tv9 stop matmuls remain at
                # the very end
                proj_out_pass1a(1, pool=ps_tv, tag="tv")
                proj_out_pass1b(1)
            else:
                if idx >= GLAG:
                    finalize(CT_ORDER[idx - GLAG])
                state[ct] = products(ct, idx)

        if DEBUG_DUMP:
            for i in range(NCT):
                sl = slice(BPC * NIJ * i, BPC * NIJ * (i + 1))
                nc.sync.dma_start(
                    dbg_h[:, sl],
                    h_sb[i][:].rearrange("p b i j -> p (b i j)"))
                nc.sync.dma_start(dbg_tv[:, sl], tvacc[i][:])

        # ---------------- proj_out epilogue ----------------
        proj_out_pass2(0)
        proj_out_pass2(1)

    nc.compile()
    return nc


# channel map: padded slot (ct, cc) -> raw channel or -1
def _chan_map():
    m = np.full(CHP, -1, np.int64)
    for ct in range(5):
        m[128 * ct:128 * (ct + 1)] = np.arange(128 * ct, 128 * (ct + 1))
    for ct in range(5, 10):
        m[128 * ct:128 * (ct + 1)] = np.arange(
            HID + 128 * (ct - 5), HID + 128 * (ct - 4))
    m[1280:1320] = np.arange(640, 680)          # x1 tail
    m[1320:1360] = np.arange(HID + 640, HID + 680)  # x2 tail
    return m


def _host_wgt(inputs):
    """fp32 numpy eval of the whole weight path (3-conv LN head + final
    conv); returns wgt packed (128, NCT*9*196) bf16 in the padded
    channel-tile layout."""
    posi = np.asarray(inputs["posi_map"], np.float32)[0]       # (4,14,14)
    x = posi
    for wk, gk, bk in (("w0", "g0", "b0"), ("w1", "g1", "b1"),
                       ("w2", "g2", "b2")):
        w = np.asarray(inputs[wk], np.float32)
        g = np.asarray(inputs[gk], np.float32)
        b = np.asarray(inputs[bk], np.float32)
        C = x.shape[0]
        xp = np.zeros((C, HP + 2, HP + 2), np.float32)
        xp[:, 1:15, 1:15] = x
        P = np.empty((C, 3, 3, NIJ), np.float32)
        for di in range(3):
            for dj in range(3):
                P[:, di, dj, :] = xp[:, di:di + HP, dj:dj + HP].reshape(C, NIJ)
        y = (w.reshape(INTER, C * 9) @ P.reshape(C * 9, NIJ))
        y = y.reshape(INTER, HP, HP)
        mu = y.mean()
        var = y.var()
        y = (y - mu) / np.sqrt(var + EPS) * g + b
        x = np.maximum(y, 0.0)
    h3p = np.zeros((INTER, HP + 2, HP + 2), np.float32)
    h3p[:, 1:15, 1:15] = x
    p3 = np.empty((576, NIJ), np.float32)
    for kap in range(NKPL):
        di, dj = kap // 3, kap % 3
        p3[kap * INTER:(kap + 1) * INTER] = \
            h3p[:, di:di + HP, dj:dj + HP].reshape(INTER, NIJ)

    # final conv as gemm: wgt[c, kpl, ij] = sum_r wfT[r, kpl, c] p3[r, ij]
    wf = np.asarray(inputs["wf"], np.float32)
    wf5 = wf.reshape(CH, NKPL, INTER, 3, 3)
    wfT = wf5.transpose(3, 4, 2, 1, 0).reshape(576, NKPL, CH)
    wgt = np.tensordot(wfT, p3, axes=(0, 0))    # (NKPL, CH, NIJ)
    wgt = wgt.transpose(1, 0, 2)                # (CH, NKPL, NIJ)

    cmap = _chan_map()
    valid = cmap >= 0
    wgtPad = np.zeros((CHP, NKPL, NIJ), np.float32)
    wgtPad[valid] = wgt[cmap[valid]]
    wgtPad = wgtPad.reshape(NCT, 128, WGT_CT).transpose(1, 0, 2)
    return np.ascontiguousarray(
        wgtPad.reshape(128, NCT * WGT_CT)).astype(ml_dtypes.bfloat16)


def _pack_shared(inputs):
    W_in = np.asarray(inputs["W_in"], np.float32)
    W_out = np.asarray(inputs["W_out"], np.float32)
    cmap = _chan_map()
    valid = cmap >= 0

    winP = np.zeros((CHP, DIM), np.float32)
    winP[valid] = W_in[cmap[valid]]
    winT = np.ascontiguousarray(winP.T).astype(ml_dtypes.bfloat16)
    # x is appended per core in kernel() to form winxT

    # W_out stationary tiles: (128, 6, 256); tile kt<5 partitions p = gated
    # channel 128*kt+p; tile 5 partitions 0:40 = channels 640:680
    woP = np.zeros((128, 6, DIM), np.float32)
    for kt in range(5):
        woP[:, kt, :] = W_out[:, 128 * kt:128 * (kt + 1)].T
    woP[0:40, 5, :] = W_out[:, 640:680].T
    woutD = woP.astype(ml_dtypes.bfloat16)

    identD = np.eye(128, dtype=np.float32).astype(ml_dtypes.bfloat16)

    return dict(winT=winT, wgtD=_host_wgt(inputs), woutD=woutD,
                identD=identD)


def kernel(**inputs) -> np.ndarray:
    if "nc" not in _CACHE:
        _CACHE["nc"] = _build_nc()
    nc = _CACHE["nc"]

    x = np.asarray(inputs["x"], np.float32)     # (64, 256, 14, 14)
    shared = _pack_shared(inputs)

    in_maps = []
    for c in range(NCORES):
        xc = x[BPC * c:BPC * (c + 1)]           # (8, 256, 14, 14)
        xT = np.ascontiguousarray(
            xc.transpose(1, 0, 2, 3).reshape(DIM, BPC * NIJ)
        ).astype(ml_dtypes.bfloat16)
        m = dict(shared)
        winT = m.pop("winT")
        m["winxT"] = np.ascontiguousarray(
            np.concatenate([winT, xT], axis=1))
        in_maps.append(m)

    res = run_bass_kernel_spmd(nc, in_maps, list(range(NCORES)))
    outs = []
    for c in range(NCORES):
        o = np.asarray(res.results[c]["out_f"], np.float32)
        o = o.reshape(DIM, BPC, HP, HP)
        outs.append(o.transpose(1, 0, 2, 3))
    return np.ascontiguousarray(np.concatenate(outs, axis=0), dtype=np.float32)
